# revision 13
# baseline (speedup 1.0000x reference)
"""Multi-head attention TRN2 Bass kernel.

Problem: B=8, S=2048, D=512, H=8 heads (head_dim 64), fp32.
  q = query @ Wq ; k = key @ Wk ; v = value @ Wv        (no bias)
  scores = softmax(q k^T / sqrt(64))  per head
  out = (scores v) @ Wo + bo

Sharding: data-parallel over batch across the 8 NeuronCores (one batch
element per core, weights replicated). No collectives; outputs stacked
host-side.

Per-core layout:
  - query/key/value loaded seq-major, PE-transposed to feature-major
    xT [D, S] (fp32 has no DMA-transpose path).
  - qT, kT feature-major [D, S]; v seq-major [S, D] with a per-head ones
    column (65 cols/head) so P@V also emits softmax denominators (row 64).
  - Heads processed in pairs: the two K=64 score matmuls go to PE row
    groups (0,0)/(64,0) and run concurrently in the array.
  - exp on the scalar engine straight out of PSUM, [128, 1024] per op,
    scale=1/8 folded in.
  - PV psum is evacuated whole [65, CW] to SBUF immediately (frees the
    psum accumulator); denominators then go partition64 -> partition0
    (SBUF-SBUF DMA) -> gpsimd partition_broadcast -> DVE
    reciprocal_approx_fast -> DVE multiply normalizes.
  - normalized heads are packed into [128, CW] pair tiles (odd head hops
    partitions via SBUF-SBUF DMA) so the output projection runs K=128;
    bias is added by DVE during the PSUM evacuation.
  - all matmuls in float32r (full PE rate at N=512).
"""

import numpy as np

_CACHE = {}

S = 2048
D = 512
H = 8
DH = 64
ST = S // 128   # 16 seq (sk) tiles
DT = D // 128   # 4 feature tiles
CW = 1024       # attention sq chunk width
NCH = S // CW   # chunks
VP = DH + 1     # 65 cols per head in padded v
NQ = CW // 512  # matmuls per chunk row


def _build():
    import concourse.bacc as bacc
    import concourse.mybir as mybir
    import concourse.tile as tile
    from concourse.masks import make_identity

    f32 = mybir.dt.float32
    f32r = mybir.dt.float32r
    bf16 = mybir.dt.bfloat16
    Exp = mybir.ActivationFunctionType.Exp

    nc = bacc.Bacc("TRN2", target_bir_lowering=False, debug=False)

    query = nc.dram_tensor("query", [S, D], f32r, kind="ExternalInput").ap()
    key = nc.dram_tensor("key", [S, D], f32r, kind="ExternalInput").ap()
    value = nc.dram_tensor("value", [S, D], f32r, kind="ExternalInput").ap()
    Wq = nc.dram_tensor("Wq", [D, D], f32r, kind="ExternalInput").ap()
    Wk = nc.dram_tensor("Wk", [D, D], f32r, kind="ExternalInput").ap()
    Wv = nc.dram_tensor("Wv", [D, D], f32r, kind="ExternalInput").ap()
    Wo = nc.dram_tensor("Wo", [D, D], f32r, kind="ExternalInput").ap()
    bo = nc.dram_tensor("bo", [1, D], f32, kind="ExternalInput").ap()
    out = nc.dram_tensor("out", [S, D], f32, kind="ExternalOutput").ap()

    with tile.TileContext(nc) as tc:
        with (
            tc.tile_pool(name="const", bufs=1) as const_pool,
            tc.tile_pool(name="wo", bufs=1) as wo_pool,
            tc.tile_pool(name="qkt", bufs=1) as qkt_pool,
            tc.tile_pool(name="vpad", bufs=1) as vpad_pool,
            tc.tile_pool(name="attn", bufs=1) as attn_pool,
        ):
            identity_f = const_pool.tile([128, 128], f32, name="identity_f")
            make_identity(nc, identity_f)
            identity = const_pool.tile([128, 128], f32r, name="identity")
            nc.vector.tensor_copy(identity[:], identity_f[:])
            ones_f = const_pool.tile([128, 128], f32, name="ones_f")
            nc.gpsimd.memset(ones_f[:], 1.0)
            onescol = const_pool.tile([128, H], f32r, name="onescol")
            nc.vector.tensor_copy(onescol[:], ones_f[:, 0:H])
            bo_sb = const_pool.tile([1, D], f32, name="bo_sb")
            nc.sync.dma_start(bo_sb[:], bo[:, :])
            bo_bc = const_pool.tile([128, D], f32, name="bo_bc")
            nc.gpsimd.partition_broadcast(bo_bc[:], bo_sb[0:1, :])

            # Wo as 4 k-tiles [128, 512]
            wo_sb = wo_pool.tile([128, D * DT], f32r, name="Wo_sb")
            for k in range(DT):
                nc.sync.dma_start(
                    wo_sb[:, k * D:(k + 1) * D], Wo[k * 128:(k + 1) * 128, :]
                )

            qT = [qkt_pool.tile([128, S], f32r, name=f"qT{j}") for j in range(DT)]
            kT = [qkt_pool.tile([128, S], f32r, name=f"kT{j}") for j in range(DT)]
            v_pad = [vpad_pool.tile([128, H * VP], f32r, name=f"vp{i}")
                     for i in range(ST)]
            # normalized attn, head pairs packed [128, CW]
            attnP = [attn_pool.tile([128, CW], f32r, name=f"attnP{g}")
                     for g in range(H // 2)]

            def load_weight(pool, wap, name):
                t = pool.tile([128, D * DT], f32r, name=name)
                for k in range(DT):
                    nc.sync.dma_start(
                        t[:, k * D:(k + 1) * D], wap[k * 128:(k + 1) * 128, :]
                    )
                return t

            def load_transposed(pool, x_dram, name):
                """4 feature-major tiles [128, S] of x^T."""
                xt = [pool.tile([128, S], f32r, name=f"{name}T{j}")
                      for j in range(DT)]
                with (
                    tc.tile_pool(name="xseq", bufs=6) as xseq_pool,
                    tc.tile_pool(name="trps", bufs=4, space="PSUM") as tr_pool,
                ):
                    for g in range(4):
                        seq_tiles = []
                        for i in range(4):
                            st = g * 4 + i
                            xt_in = xseq_pool.tile([128, D], f32r, tag="xseq")
                            nc.sync.dma_start(
                                xt_in[:], x_dram[st * 128:(st + 1) * 128, :]
                            )
                            seq_tiles.append(xt_in)
                        for j in range(DT):
                            ps = tr_pool.tile([128, 512], f32r, tag="trps")
                            for i in range(4):
                                nc.tensor.transpose(
                                    ps[:, i * 128:(i + 1) * 128],
                                    seq_tiles[i][:, j * 128:(j + 1) * 128],
                                    identity[:],
                                )
                            nc.vector.tensor_copy(
                                xt[j][:, g * 512:(g + 1) * 512], ps[:]
                            )
                return xt

            def project_T(xt, w, dst):
                """dst[j] = (x @ W)^T  (feature-major)."""
                with tc.tile_pool(name="pps", bufs=4, space="PSUM") as pps:
                    for j in range(DT):
                        for g in range(4):
                            ps = pps.tile([128, 512], f32, tag="pps")
                            for k in range(DT):
                                nc.tensor.matmul(
                                    ps[:],
                                    w[:, k * D + j * 128: k * D + (j + 1) * 128],
                                    xt[k][:, g * 512:(g + 1) * 512],
                                    start=(k == 0),
                                    stop=(k == DT - 1),
                                )
                            nc.vector.tensor_copy(
                                dst[j][:, g * 512:(g + 1) * 512], ps[:]
                            )

            # ---- transposed loads + projections ----
            with tc.tile_pool(name="xtq", bufs=1) as xtq_pool:
                wq_sb = load_weight(xtq_pool, Wq, "Wq_sb")
                queryT = load_transposed(xtq_pool, query, "query")
                project_T(queryT, wq_sb, qT)
            with tc.tile_pool(name="xtk", bufs=1) as xtk_pool:
                wk_sb = load_weight(xtk_pool, Wk, "Wk_sb")
                keyT = load_transposed(xtk_pool, key, "key")
                project_T(keyT, wk_sb, kT)
            with tc.tile_pool(name="xtv", bufs=1) as xtv_pool:
                wv_sb = load_weight(xtv_pool, Wv, "Wv_sb")
                valueT = load_transposed(xtv_pool, value, "value")
                with tc.tile_pool(name="vps", bufs=4, space="PSUM") as vps:
                    for st in range(ST):
                        ps = vps.tile([128, 512], f32, tag="vps")
                        for k in range(DT):
                            nc.tensor.matmul(
                                ps[:],
                                valueT[k][:, st * 128:(st + 1) * 128],
                                wv_sb[:, k * D:(k + 1) * D],
                                start=(k == 0),
                                stop=(k == DT - 1),
                            )
                        dst = v_pad[st].rearrange("p (h c) -> p h c", c=VP)
                        nc.vector.tensor_copy(
                            dst[:, :, 0:DH],
                            ps.rearrange("p (h c) -> p h c", c=DH),
                        )
                        nc.vector.tensor_copy(
                            dst[:, :, DH:VP],
                            onescol.rearrange("p (h o) -> p h o", o=1),
                        )

            # ---- attention (head pairs) + output projection per chunk ----
            with (
                tc.tile_pool(name="scp", bufs=1, space="PSUM") as scp,
                tc.tile_pool(name="pvp0", bufs=1, space="PSUM") as pvp0,
                tc.tile_pool(name="pvp1", bufs=1, space="PSUM") as pvp1,
                tc.tile_pool(name="expp", bufs=2) as expp,
                tc.tile_pool(name="smalls", bufs=2) as smalls,
                tc.tile_pool(name="outp", bufs=2) as outp,
            ):
                for c in range(NCH):
                    for g in range(H // 2):
                        h0, h1 = 2 * g, 2 * g + 1
                        jj = g
                        pv0 = pvp0.tile([128, CW], f32, tag="pv0")
                        pv1 = pvp1.tile([128, CW], f32, tag="pv1")
                        for st in range(ST):
                            sc = scp.tile([128, 2 * CW], f32, tag="sc")
                            for q2 in range(NQ):
                                sq0 = c * CW + q2 * 512
                                qs0 = slice(q2 * 512, (q2 + 1) * 512)
                                qs1 = slice(CW + q2 * 512, CW + (q2 + 1) * 512)
                                ks = slice(st * 128, (st + 1) * 128)
                                nc.tensor.matmul(
                                    sc[:, qs0], kT[jj][0:DH, ks],
                                    qT[jj][0:DH, sq0:sq0 + 512],
                                    start=True, stop=True,
                                    tile_position=(0, 0),
                                )
                                nc.tensor.matmul(
                                    sc[:, qs1], kT[jj][DH:128, ks],
                                    qT[jj][DH:128, sq0:sq0 + 512],
                                    start=True, stop=True,
                                    tile_position=(DH, 0),
                                )
                            ex = expp.tile([128, 2 * CW], f32r, tag="ex")
                            nc.scalar.activation(ex[:], sc[:], Exp, scale=0.125)
                            for q2 in range(NQ):
                                qs0 = slice(q2 * 512, (q2 + 1) * 512)
                                qs1 = slice(CW + q2 * 512, CW + (q2 + 1) * 512)
                                nc.tensor.matmul(
                                    pv0[0:VP, qs0],
                                    v_pad[st][:, h0 * VP:(h0 + 1) * VP],
                                    ex[:, qs0],
                                    start=(st == 0), stop=(st == ST - 1),
                                )
                                nc.tensor.matmul(
                                    pv1[0:VP, qs0],
                                    v_pad[st][:, h1 * VP:(h1 + 1) * VP],
                                    ex[:, qs1],
                                    start=(st == 0), stop=(st == ST - 1),
                                )
                        # normalize both heads; h0 -> attnP[g][0:64],
                        # h1 -> staging -> DMA hop to attnP[g][64:128]
                        for hh, pv in ((0, pv0), (1, pv1)):
                            un = smalls.tile([128, CW], f32, tag="un")
                            nc.vector.tensor_copy(un[0:VP, :], pv[0:VP, :])
                            rsrc = smalls.tile([1, CW], f32, tag="rsrc")
                            nc.sync.dma_start(rsrc[0:1, :], un[DH:VP, :])
                            rpre = smalls.tile([DH, CW], f32, tag="rpre")
                            nc.gpsimd.partition_broadcast(
                                rpre[:], rsrc[0:1, :]
                            )
                            rec = smalls.tile([DH, CW], f32, tag="rec")
                            nc.vector.reciprocal_approx_fast(rec[:], rpre[:])
                            if hh == 0:
                                nc.vector.tensor_mul(
                                    attnP[g][0:DH, :], un[0:DH, :], rec[:]
                                )
                            else:
                                stg = smalls.tile([DH, CW], f32r, tag="stg")
                                nc.vector.tensor_mul(
                                    stg[:], un[0:DH, :], rec[:]
                                )
                                nc.sync.dma_start(
                                    attnP[g][DH:128, :], stg[:]
                                )

                    # output projection for this chunk
                    for t2 in range(CW // 128):
                        t = c * (CW // 128) + t2
                        ops = (pvp0 if t2 % 2 == 0 else pvp1).tile(
                            [128, D], f32, tag=("pv0" if t2 % 2 == 0 else "pv1")
                        )
                        for g in range(H // 2):
                            nc.tensor.matmul(
                                ops[:],
                                attnP[g][:, t2 * 128:(t2 + 1) * 128],
                                wo_sb[:, g * D:(g + 1) * D],
                                start=(g == 0), stop=(g == H // 2 - 1),
                            )
                        ot = outp.tile([128, D], f32, tag="ot")
                        nc.vector.tensor_add(ot[:], ops[:], bo_bc[:])
                        nc.sync.dma_start(out[t * 128:(t + 1) * 128, :], ot[:])

    nc.compile()
    return nc


def _get_nc():
    if "nc" not in _CACHE:
        _CACHE["nc"] = _build()
    return _CACHE["nc"]


def kernel(query, key, value, Wq, Wk, Wv, Wo, bo):
    from concourse.bass_utils import run_bass_kernel_spmd

    nc = _get_nc()
    B = query.shape[0]
    assert B == 8
    to_np = lambda a: np.asarray(a, dtype=np.float32)
    Wq, Wk, Wv, Wo = to_np(Wq), to_np(Wk), to_np(Wv), to_np(Wo)
    bo2 = to_np(bo).reshape(1, D)
    in_maps = [
        {
            "query": to_np(query[b]),
            "key": to_np(key[b]),
            "value": to_np(value[b]),
            "Wq": Wq, "Wk": Wk, "Wv": Wv, "Wo": Wo, "bo": bo2,
        }
        for b in range(B)
    ]
    res = run_bass_kernel_spmd(nc, in_maps, list(range(B)))
    return np.stack([res.results[b]["out"] for b in range(B)], axis=0)


# revision 14
# speedup vs baseline: 1.7501x; 1.7501x over previous
"""Multi-head attention TRN2 Bass kernel.

Problem: B=8, S=2048, D=512, H=8 heads (head_dim 64), fp32.
  q = query @ Wq ; k = key @ Wk ; v = value @ Wv        (no bias)
  scores = softmax(q k^T / sqrt(64))  per head
  out = (scores v) @ Wo + bo

Sharding: data-parallel over batch across the 8 NeuronCores (one batch
element per core, weights replicated). No collectives; outputs stacked
host-side.

Per-core layout:
  - query/key/value loaded seq-major, PE-transposed to feature-major
    xT [D, S] (fp32 has no DMA-transpose path).
  - qT, kT feature-major [D, S]; v seq-major [S, D] with a per-head ones
    column (65 cols/head) so P@V also emits softmax denominators (row 64).
  - Heads processed in pairs: the two K=64 score matmuls go to PE row
    groups (0,0)/(64,0) and run concurrently in the array.
  - exp on the scalar engine straight out of PSUM, [128, 1024] per op,
    scale=1/8 folded in.
  - PV psum is evacuated whole [65, CW] to SBUF immediately (frees the
    psum accumulator); denominators then go partition64 -> partition0
    (SBUF-SBUF DMA) -> gpsimd partition_broadcast -> DVE
    reciprocal_approx_fast -> DVE multiply normalizes.
  - normalized heads are packed into [128, CW] pair tiles (odd head hops
    partitions via SBUF-SBUF DMA) so the output projection runs K=128;
    bias is added by DVE during the PSUM evacuation.
  - all matmuls in float32r (full PE rate at N=512).
"""

import numpy as np

_CACHE = {}

S = 2048
D = 512
H = 8
DH = 64
ST = S // 128   # 16 seq (sk) tiles
DT = D // 128   # 4 feature tiles
CW = 512        # attention sq chunk width
NCH = S // CW   # chunks
VP = DH + 1     # 65 cols per head in padded v
NQ = CW // 512  # matmuls per chunk row


def _build():
    import concourse.bacc as bacc
    import concourse.mybir as mybir
    import concourse.tile as tile
    from concourse.masks import make_identity

    f32 = mybir.dt.float32
    f32r = mybir.dt.float32r
    bf16 = mybir.dt.bfloat16
    Exp = mybir.ActivationFunctionType.Exp

    nc = bacc.Bacc("TRN2", target_bir_lowering=False, debug=False)

    query = nc.dram_tensor("query", [S, D], f32r, kind="ExternalInput").ap()
    key = nc.dram_tensor("key", [S, D], f32r, kind="ExternalInput").ap()
    value = nc.dram_tensor("value", [S, D], f32r, kind="ExternalInput").ap()
    Wq = nc.dram_tensor("Wq", [D, D], f32r, kind="ExternalInput").ap()
    Wk = nc.dram_tensor("Wk", [D, D], f32r, kind="ExternalInput").ap()
    Wv = nc.dram_tensor("Wv", [D, D], f32r, kind="ExternalInput").ap()
    Wo = nc.dram_tensor("Wo", [D, D], f32r, kind="ExternalInput").ap()
    bo = nc.dram_tensor("bo", [1, D], f32, kind="ExternalInput").ap()
    out = nc.dram_tensor("out", [S, D], f32, kind="ExternalOutput").ap()

    with tile.TileContext(nc) as tc:
        with (
            tc.tile_pool(name="const", bufs=1) as const_pool,
            tc.tile_pool(name="wo", bufs=1) as wo_pool,
            tc.tile_pool(name="qkt", bufs=1) as qkt_pool,
            tc.tile_pool(name="vpad", bufs=1) as vpad_pool,
            tc.tile_pool(name="attn", bufs=1) as attn_pool,
        ):
            identity_f = const_pool.tile([128, 128], f32, name="identity_f")
            make_identity(nc, identity_f)
            identity = const_pool.tile([128, 128], f32r, name="identity")
            nc.vector.tensor_copy(identity[:], identity_f[:])
            ones_f = const_pool.tile([128, 128], f32, name="ones_f")
            nc.gpsimd.memset(ones_f[:], 1.0)
            onescol = const_pool.tile([128, H], f32r, name="onescol")
            nc.vector.tensor_copy(onescol[:], ones_f[:, 0:H])
            bo_sb = const_pool.tile([1, D], f32, name="bo_sb")
            nc.sync.dma_start(bo_sb[:], bo[:, :])
            bo_bc = const_pool.tile([128, D], f32, name="bo_bc")
            nc.gpsimd.partition_broadcast(bo_bc[:], bo_sb[0:1, :])

            # Wo as 4 k-tiles [128, 512]
            wo_sb = wo_pool.tile([128, D * DT], f32r, name="Wo_sb")
            for k in range(DT):
                nc.sync.dma_start(
                    wo_sb[:, k * D:(k + 1) * D], Wo[k * 128:(k + 1) * 128, :]
                )

            qT = [qkt_pool.tile([128, S], f32r, name=f"qT{j}") for j in range(DT)]
            kT = [qkt_pool.tile([128, S], f32r, name=f"kT{j}") for j in range(DT)]
            v_pad = [vpad_pool.tile([128, H * VP], f32r, name=f"vp{i}")
                     for i in range(ST)]
            # normalized attn, head pairs packed [128, CW]
            attnP = [attn_pool.tile([128, CW], f32r, name=f"attnP{g}")
                     for g in range(H // 2)]

            def load_weight(pool, wap, name):
                t = pool.tile([128, D * DT], f32r, name=name)
                for k in range(DT):
                    nc.sync.dma_start(
                        t[:, k * D:(k + 1) * D], wap[k * 128:(k + 1) * 128, :]
                    )
                return t

            def load_transposed(pool, x_dram, name):
                """4 feature-major tiles [128, S] of x^T."""
                xt = [pool.tile([128, S], f32r, name=f"{name}T{j}")
                      for j in range(DT)]
                with (
                    tc.tile_pool(name="xseq", bufs=6) as xseq_pool,
                    tc.tile_pool(name="trps", bufs=4, space="PSUM") as tr_pool,
                ):
                    for g in range(4):
                        seq_tiles = []
                        for i in range(4):
                            st = g * 4 + i
                            xt_in = xseq_pool.tile([128, D], f32r, tag="xseq")
                            nc.sync.dma_start(
                                xt_in[:], x_dram[st * 128:(st + 1) * 128, :]
                            )
                            seq_tiles.append(xt_in)
                        for j in range(DT):
                            ps = tr_pool.tile([128, 512], f32r, tag="trps")
                            for i in range(4):
                                nc.tensor.transpose(
                                    ps[:, i * 128:(i + 1) * 128],
                                    seq_tiles[i][:, j * 128:(j + 1) * 128],
                                    identity[:],
                                )
                            nc.vector.tensor_copy(
                                xt[j][:, g * 512:(g + 1) * 512], ps[:]
                            )
                return xt

            def project_T(xt, w, dst):
                """dst[j] = (x @ W)^T  (feature-major)."""
                with tc.tile_pool(name="pps", bufs=4, space="PSUM") as pps:
                    for j in range(DT):
                        for g in range(4):
                            ps = pps.tile([128, 512], f32, tag="pps")
                            for k in range(DT):
                                nc.tensor.matmul(
                                    ps[:],
                                    w[:, k * D + j * 128: k * D + (j + 1) * 128],
                                    xt[k][:, g * 512:(g + 1) * 512],
                                    start=(k == 0),
                                    stop=(k == DT - 1),
                                )
                            nc.vector.tensor_copy(
                                dst[j][:, g * 512:(g + 1) * 512], ps[:]
                            )

            # ---- transposed loads + projections ----
            with tc.tile_pool(name="xtq", bufs=1) as xtq_pool:
                wq_sb = load_weight(xtq_pool, Wq, "Wq_sb")
                queryT = load_transposed(xtq_pool, query, "query")
                project_T(queryT, wq_sb, qT)
            with tc.tile_pool(name="xtk", bufs=1) as xtk_pool:
                wk_sb = load_weight(xtk_pool, Wk, "Wk_sb")
                keyT = load_transposed(xtk_pool, key, "key")
                project_T(keyT, wk_sb, kT)
            with tc.tile_pool(name="xtv", bufs=1) as xtv_pool:
                wv_sb = load_weight(xtv_pool, Wv, "Wv_sb")
                valueT = load_transposed(xtv_pool, value, "value")
                with tc.tile_pool(name="vps", bufs=4, space="PSUM") as vps:
                    for st in range(ST):
                        ps = vps.tile([128, 512], f32, tag="vps")
                        for k in range(DT):
                            nc.tensor.matmul(
                                ps[:],
                                valueT[k][:, st * 128:(st + 1) * 128],
                                wv_sb[:, k * D:(k + 1) * D],
                                start=(k == 0),
                                stop=(k == DT - 1),
                            )
                        dst = v_pad[st].rearrange("p (h c) -> p h c", c=VP)
                        nc.vector.tensor_copy(
                            dst[:, :, 0:DH],
                            ps.rearrange("p (h c) -> p h c", c=DH),
                        )
                        nc.vector.tensor_copy(
                            dst[:, :, DH:VP],
                            onescol.rearrange("p (h o) -> p h o", o=1),
                        )

            # ---- attention (head pairs) + output projection per chunk ----
            with (
                tc.tile_pool(name="scp", bufs=2, space="PSUM") as scp,
                tc.tile_pool(name="pvp0", bufs=1, space="PSUM") as pvp0,
                tc.tile_pool(name="pvp1", bufs=1, space="PSUM") as pvp1,
                tc.tile_pool(name="oup", bufs=2, space="PSUM") as oup,
                tc.tile_pool(name="expp", bufs=3) as expp,
                tc.tile_pool(name="smalls", bufs=2) as smalls,
                tc.tile_pool(name="outp", bufs=2) as outp,
            ):
                def warm_blast(n):
                    # dense back-to-back matmuls to trip the PE clock gate
                    wps = oup.tile([128, D], f32, tag="ops")
                    for i in range(n):
                        nc.tensor.matmul(
                            wps[:], wo_sb[:, 0:128], qT[0][:, 0:D],
                            start=(i == 0), stop=(i == n - 1),
                        )

                warm_blast(16)
                for c in range(NCH):
                    for g in range(H // 2):
                        h0, h1 = 2 * g, 2 * g + 1
                        jj = g
                        pv0 = pvp0.tile([VP, CW], f32, tag="pv0")
                        pv1 = pvp1.tile([VP, CW], f32, tag="pv1")
                        for st in range(ST):
                            sq0 = c * CW
                            ks = slice(st * 128, (st + 1) * 128)
                            sc = scp.tile([128, 2 * CW], f32, tag="sc")
                            nc.tensor.matmul(
                                sc[:, 0:CW], kT[jj][0:DH, ks],
                                qT[jj][0:DH, sq0:sq0 + CW],
                                start=True, stop=True,
                                tile_position=(0, 0),
                            )
                            nc.tensor.matmul(
                                sc[:, CW:2 * CW], kT[jj][DH:128, ks],
                                qT[jj][DH:128, sq0:sq0 + CW],
                                start=True, stop=True,
                                tile_position=(DH, 0),
                            )
                            ex = expp.tile([128, 2 * CW], f32r, tag="ex")
                            nc.scalar.activation(ex[:], sc[:], Exp, scale=0.125)
                            nc.tensor.matmul(
                                pv0[0:VP, :],
                                v_pad[st][:, h0 * VP:(h0 + 1) * VP],
                                ex[:, 0:CW],
                                start=(st == 0), stop=(st == ST - 1),
                            )
                            nc.tensor.matmul(
                                pv1[0:VP, :],
                                v_pad[st][:, h1 * VP:(h1 + 1) * VP],
                                ex[:, CW:2 * CW],
                                start=(st == 0), stop=(st == ST - 1),
                            )
                        # normalize both heads; h0 -> attnP[g][0:64],
                        # h1 -> staging -> DMA hop to attnP[g][64:128]
                        for hh, pv in ((0, pv0), (1, pv1)):
                            un = smalls.tile([128, CW], f32, tag="un")
                            nc.vector.tensor_copy(un[0:VP, :], pv[0:VP, :])
                            rsrc = smalls.tile([1, CW], f32, tag="rsrc")
                            nc.sync.dma_start(rsrc[0:1, :], un[DH:VP, :])
                            rpre = smalls.tile([DH, CW], f32, tag="rpre")
                            nc.gpsimd.partition_broadcast(
                                rpre[:], rsrc[0:1, :]
                            )
                            rec = smalls.tile([DH, CW], f32, tag="rec")
                            nc.vector.reciprocal_approx_fast(rec[:], rpre[:])
                            if hh == 0:
                                nc.vector.tensor_mul(
                                    attnP[g][0:DH, :], un[0:DH, :], rec[:]
                                )
                            else:
                                stg = smalls.tile([DH, CW], f32r, tag="stg")
                                nc.vector.tensor_mul(
                                    stg[:], un[0:DH, :], rec[:]
                                )
                                nc.sync.dma_start(
                                    attnP[g][DH:128, :], stg[:]
                                )

                    # output projection for this chunk
                    for t2 in range(CW // 128):
                        t = c * (CW // 128) + t2
                        ops = oup.tile([128, D], f32, tag="ops")
                        for g in range(H // 2):
                            nc.tensor.matmul(
                                ops[:],
                                attnP[g][:, t2 * 128:(t2 + 1) * 128],
                                wo_sb[:, g * D:(g + 1) * D],
                                start=(g == 0), stop=(g == H // 2 - 1),
                            )
                        ot = outp.tile([128, D], f32, tag="ot")
                        nc.vector.tensor_add(ot[:], ops[:], bo_bc[:])
                        nc.sync.dma_start(out[t * 128:(t + 1) * 128, :], ot[:])

    nc.compile()
    return nc


def _get_nc():
    if "nc" not in _CACHE:
        _CACHE["nc"] = _build()
    return _CACHE["nc"]


def kernel(query, key, value, Wq, Wk, Wv, Wo, bo):
    from concourse.bass_utils import run_bass_kernel_spmd

    nc = _get_nc()
    B = query.shape[0]
    assert B == 8
    to_np = lambda a: np.asarray(a, dtype=np.float32)
    Wq, Wk, Wv, Wo = to_np(Wq), to_np(Wk), to_np(Wv), to_np(Wo)
    bo2 = to_np(bo).reshape(1, D)
    in_maps = [
        {
            "query": to_np(query[b]),
            "key": to_np(key[b]),
            "value": to_np(value[b]),
            "Wq": Wq, "Wk": Wk, "Wv": Wv, "Wo": Wo, "bo": bo2,
        }
        for b in range(B)
    ]
    res = run_bass_kernel_spmd(nc, in_maps, list(range(B)))
    return np.stack([res.results[b]["out"] for b in range(B)], axis=0)


# revision 16
# speedup vs baseline: 1.7779x; 1.0159x over previous
"""Multi-head attention TRN2 Bass kernel.

Problem: B=8, S=2048, D=512, H=8 heads (head_dim 64), fp32.
  q = query @ Wq ; k = key @ Wk ; v = value @ Wv        (no bias)
  scores = softmax(q k^T / sqrt(64))  per head
  out = (scores v) @ Wo + bo

Sharding: data-parallel over batch across the 8 NeuronCores (one batch
element per core, weights replicated). No collectives; outputs stacked
host-side.

Per-core layout:
  - query/key/value loaded seq-major, PE-transposed to feature-major
    xT [D, S] (fp32 has no DMA-transpose path).
  - qT, kT feature-major [D, S]; v seq-major [S, D] with a per-head ones
    column (65 cols/head) so P@V also emits softmax denominators (row 64).
  - Heads processed in pairs: the two K=64 score matmuls go to PE row
    groups (0,0)/(64,0) and run concurrently in the array.
  - exp on the scalar engine straight out of PSUM, [128, 1024] per op,
    scale=1/8 folded in.
  - PV psum is evacuated whole [65, CW] to SBUF immediately (frees the
    psum accumulator); denominators then go partition64 -> partition0
    (SBUF-SBUF DMA) -> gpsimd partition_broadcast -> DVE
    reciprocal_approx_fast -> DVE multiply normalizes.
  - normalized heads are packed into [128, CW] pair tiles (odd head hops
    partitions via SBUF-SBUF DMA) so the output projection runs K=128;
    bias is added by DVE during the PSUM evacuation.
  - all matmuls in float32r (full PE rate at N=512).
"""

import numpy as np

_CACHE = {}

S = 2048
D = 512
H = 8
DH = 64
ST = S // 128   # 16 seq (sk) tiles
DT = D // 128   # 4 feature tiles
CW = 512        # attention sq chunk width
NCH = S // CW   # chunks
VP = DH + 1     # 65 cols per head in padded v
NQ = CW // 512  # matmuls per chunk row


def _build():
    import concourse.bacc as bacc
    import concourse.mybir as mybir
    import concourse.tile as tile
    from concourse.masks import make_identity

    f32 = mybir.dt.float32
    f32r = mybir.dt.float32r
    bf16 = mybir.dt.bfloat16
    Exp = mybir.ActivationFunctionType.Exp

    nc = bacc.Bacc("TRN2", target_bir_lowering=False, debug=False)

    query = nc.dram_tensor("query", [S, D], f32r, kind="ExternalInput").ap()
    key = nc.dram_tensor("key", [S, D], f32r, kind="ExternalInput").ap()
    value = nc.dram_tensor("value", [S, D], f32r, kind="ExternalInput").ap()
    Wq = nc.dram_tensor("Wq", [D, D], f32r, kind="ExternalInput").ap()
    Wk = nc.dram_tensor("Wk", [D, D], f32r, kind="ExternalInput").ap()
    Wv = nc.dram_tensor("Wv", [D, D], f32r, kind="ExternalInput").ap()
    Wo = nc.dram_tensor("Wo", [D, D], f32r, kind="ExternalInput").ap()
    bo = nc.dram_tensor("bo", [1, D], f32, kind="ExternalInput").ap()
    out = nc.dram_tensor("out", [S, D], f32, kind="ExternalOutput").ap()

    with tile.TileContext(nc) as tc:
        with (
            tc.tile_pool(name="const", bufs=1) as const_pool,
            tc.tile_pool(name="wo", bufs=1) as wo_pool,
            tc.tile_pool(name="qkt", bufs=1) as qkt_pool,
            tc.tile_pool(name="vpad", bufs=1) as vpad_pool,
            tc.tile_pool(name="attn", bufs=1) as attn_pool,
        ):
            identity_f = const_pool.tile([128, 128], f32, name="identity_f")
            make_identity(nc, identity_f)
            identity = const_pool.tile([128, 128], f32r, name="identity")
            nc.vector.tensor_copy(identity[:], identity_f[:])
            ones_f = const_pool.tile([128, 128], f32, name="ones_f")
            nc.gpsimd.memset(ones_f[:], 1.0)
            onescol = const_pool.tile([128, H], f32r, name="onescol")
            nc.vector.tensor_copy(onescol[:], ones_f[:, 0:H])
            bo_sb = const_pool.tile([1, D], f32, name="bo_sb")
            nc.sync.dma_start(bo_sb[:], bo[:, :])
            bo_bc = const_pool.tile([128, D], f32, name="bo_bc")
            nc.gpsimd.partition_broadcast(bo_bc[:], bo_sb[0:1, :])

            # Wo as 4 k-tiles [128, 512]
            wo_sb = wo_pool.tile([128, D * DT], f32r, name="Wo_sb")
            nc.sync.dma_start(
                wo_sb.rearrange("p (a d) -> p a d", d=D),
                Wo.rearrange("(a p) d -> p a d", p=128),
            )

            qT = [qkt_pool.tile([128, S], f32r, name=f"qT{j}") for j in range(DT)]
            kT = [qkt_pool.tile([128, S], f32r, name=f"kT{j}") for j in range(DT)]
            v_pad = [vpad_pool.tile([128, H * VP], f32r, name=f"vp{i}")
                     for i in range(ST)]
            # normalized attn, head pairs packed [128, CW]
            attnP = [attn_pool.tile([128, CW], f32r, name=f"attnP{g}")
                     for g in range(H // 2)]

            def load_weight(pool, wap, name):
                t = pool.tile([128, D * DT], f32r, name=name)
                nc.sync.dma_start(
                    t.rearrange("p (a d) -> p a d", d=D),
                    wap.rearrange("(a p) d -> p a d", p=128),
                )
                return t

            def load_transposed(pool, x_dram, name):
                """4 feature-major tiles [128, S] of x^T."""
                xt = [pool.tile([128, S], f32r, name=f"{name}T{j}")
                      for j in range(DT)]
                with (
                    tc.tile_pool(name="xseq", bufs=3) as xseq_pool,
                    tc.tile_pool(name="trps", bufs=3, space="PSUM") as tr_pool,
                ):
                    for g in range(4):
                        xt_in = xseq_pool.tile([128, 4 * D], f32r, tag="xseq")
                        nc.sync.dma_start(
                            xt_in.rearrange("p (a d) -> p a d", d=D),
                            x_dram[g * 512:(g + 1) * 512, :].rearrange(
                                "(a p) d -> p a d", p=128
                            ),
                        )
                        for j in range(DT):
                            ps = tr_pool.tile([128, 512], f32r, tag="trps")
                            for i in range(4):
                                nc.tensor.transpose(
                                    ps[:, i * 128:(i + 1) * 128],
                                    xt_in[:, i * D + j * 128: i * D + (j + 1) * 128],
                                    identity[:],
                                )
                            nc.vector.tensor_copy(
                                xt[j][:, g * 512:(g + 1) * 512], ps[:]
                            )
                return xt

            def project_T(xt, w, dst):
                """dst[j] = (x @ W)^T  (feature-major)."""
                with tc.tile_pool(name="pps", bufs=4, space="PSUM") as pps:
                    for j in range(DT):
                        for g in range(4):
                            ps = pps.tile([128, 512], f32, tag="pps")
                            for k in range(DT):
                                nc.tensor.matmul(
                                    ps[:],
                                    w[:, k * D + j * 128: k * D + (j + 1) * 128],
                                    xt[k][:, g * 512:(g + 1) * 512],
                                    start=(k == 0),
                                    stop=(k == DT - 1),
                                )
                            nc.vector.tensor_copy(
                                dst[j][:, g * 512:(g + 1) * 512], ps[:]
                            )

            # ---- transposed loads + projections ----
            def blast_w(bp, w, n):
                wps = bp.tile([128, D], f32, tag="blast")
                for i in range(n):
                    nc.tensor.matmul(
                        wps[:], w[:, 0:128], w[:, 0:D],
                        start=(i == 0), stop=(i == n - 1),
                    )

            with tc.tile_pool(name="blastp", bufs=1, space="PSUM") as blastp:
                with tc.tile_pool(name="xtq", bufs=1) as xtq_pool:
                    wq_sb = load_weight(xtq_pool, Wq, "Wq_sb")
                    blast_w(blastp, wq_sb, 20)
                    queryT = load_transposed(xtq_pool, query, "query")
                    project_T(queryT, wq_sb, qT)
                with tc.tile_pool(name="xtk", bufs=1) as xtk_pool:
                    wk_sb = load_weight(xtk_pool, Wk, "Wk_sb")
                    blast_w(blastp, wk_sb, 20)
                    keyT = load_transposed(xtk_pool, key, "key")
                    project_T(keyT, wk_sb, kT)
                with tc.tile_pool(name="xtv", bufs=1) as xtv_pool:
                    wv_sb = load_weight(xtv_pool, Wv, "Wv_sb")
                    blast_w(blastp, wv_sb, 20)
                    valueT = load_transposed(xtv_pool, value, "value")
                    vps_stack = tc.tile_pool(name="vps", bufs=4, space="PSUM")
                    with vps_stack as vps:
                        for st in range(ST):
                            ps = vps.tile([128, 512], f32, tag="vps")
                            for k in range(DT):
                                nc.tensor.matmul(
                                    ps[:],
                                    valueT[k][:, st * 128:(st + 1) * 128],
                                    wv_sb[:, k * D:(k + 1) * D],
                                    start=(k == 0),
                                    stop=(k == DT - 1),
                                )
                            dst = v_pad[st].rearrange("p (h c) -> p h c", c=VP)
                            nc.vector.tensor_copy(
                                dst[:, :, 0:DH],
                                ps.rearrange("p (h c) -> p h c", c=DH),
                            )
                            nc.vector.tensor_copy(
                                dst[:, :, DH:VP],
                                onescol.rearrange("p (h o) -> p h o", o=1),
                            )

            # ---- attention (head pairs) + output projection per chunk ----
            with (
                tc.tile_pool(name="scp", bufs=2, space="PSUM") as scp,
                tc.tile_pool(name="pvp0", bufs=1, space="PSUM") as pvp0,
                tc.tile_pool(name="pvp1", bufs=1, space="PSUM") as pvp1,
                tc.tile_pool(name="oup", bufs=2, space="PSUM") as oup,
                tc.tile_pool(name="expp", bufs=3) as expp,
                tc.tile_pool(name="smalls", bufs=2) as smalls,
                tc.tile_pool(name="outp", bufs=2) as outp,
            ):
                def warm_blast(n):
                    # dense back-to-back matmuls to trip the PE clock gate
                    wps = oup.tile([128, D], f32, tag="ops")
                    for i in range(n):
                        nc.tensor.matmul(
                            wps[:], wo_sb[:, 0:128], qT[0][:, 0:D],
                            start=(i == 0), stop=(i == n - 1),
                        )

                warm_blast(16)
                for c in range(NCH):
                    for g in range(H // 2):
                        h0, h1 = 2 * g, 2 * g + 1
                        jj = g
                        pv0 = pvp0.tile([VP, CW], f32, tag="pv0")
                        pv1 = pvp1.tile([VP, CW], f32, tag="pv1")
                        for st in range(ST):
                            sq0 = c * CW
                            ks = slice(st * 128, (st + 1) * 128)
                            sc = scp.tile([128, 2 * CW], f32, tag="sc")
                            nc.tensor.matmul(
                                sc[:, 0:CW], kT[jj][0:DH, ks],
                                qT[jj][0:DH, sq0:sq0 + CW],
                                start=True, stop=True,
                                tile_position=(0, 0),
                            )
                            nc.tensor.matmul(
                                sc[:, CW:2 * CW], kT[jj][DH:128, ks],
                                qT[jj][DH:128, sq0:sq0 + CW],
                                start=True, stop=True,
                                tile_position=(DH, 0),
                            )
                            ex = expp.tile([128, 2 * CW], f32r, tag="ex")
                            nc.scalar.activation(ex[:], sc[:], Exp, scale=0.125)
                            nc.tensor.matmul(
                                pv0[0:VP, :],
                                v_pad[st][:, h0 * VP:(h0 + 1) * VP],
                                ex[:, 0:CW],
                                start=(st == 0), stop=(st == ST - 1),
                            )
                            nc.tensor.matmul(
                                pv1[0:VP, :],
                                v_pad[st][:, h1 * VP:(h1 + 1) * VP],
                                ex[:, CW:2 * CW],
                                start=(st == 0), stop=(st == ST - 1),
                            )
                        # normalize both heads; h0 -> attnP[g][0:64],
                        # h1 -> staging -> DMA hop to attnP[g][64:128]
                        for hh, pv in ((0, pv0), (1, pv1)):
                            un = smalls.tile([128, CW], f32, tag="un")
                            nc.vector.tensor_copy(un[0:VP, :], pv[0:VP, :])
                            rsrc = smalls.tile([1, CW], f32, tag="rsrc")
                            nc.sync.dma_start(rsrc[0:1, :], un[DH:VP, :])
                            rpre = smalls.tile([DH, CW], f32, tag="rpre")
                            nc.gpsimd.partition_broadcast(
                                rpre[:], rsrc[0:1, :]
                            )
                            rec = smalls.tile([DH, CW], f32, tag="rec")
                            nc.vector.reciprocal_approx_fast(rec[:], rpre[:])
                            if hh == 0:
                                nc.vector.tensor_mul(
                                    attnP[g][0:DH, :], un[0:DH, :], rec[:]
                                )
                            else:
                                stg = smalls.tile([DH, CW], f32r, tag="stg")
                                nc.vector.tensor_mul(
                                    stg[:], un[0:DH, :], rec[:]
                                )
                                nc.sync.dma_start(
                                    attnP[g][DH:128, :], stg[:]
                                )

                    # output projection for this chunk
                    for t2 in range(CW // 128):
                        t = c * (CW // 128) + t2
                        ops = oup.tile([128, D], f32, tag="ops")
                        for g in range(H // 2):
                            nc.tensor.matmul(
                                ops[:],
                                attnP[g][:, t2 * 128:(t2 + 1) * 128],
                                wo_sb[:, g * D:(g + 1) * D],
                                start=(g == 0), stop=(g == H // 2 - 1),
                            )
                        ot = outp.tile([128, D], f32, tag="ot")
                        nc.vector.tensor_add(ot[:], ops[:], bo_bc[:])
                        nc.sync.dma_start(out[t * 128:(t + 1) * 128, :], ot[:])

    nc.compile()
    return nc


def _get_nc():
    if "nc" not in _CACHE:
        _CACHE["nc"] = _build()
    return _CACHE["nc"]


def kernel(query, key, value, Wq, Wk, Wv, Wo, bo):
    from concourse.bass_utils import run_bass_kernel_spmd

    nc = _get_nc()
    B = query.shape[0]
    assert B == 8
    to_np = lambda a: np.asarray(a, dtype=np.float32)
    Wq, Wk, Wv, Wo = to_np(Wq), to_np(Wk), to_np(Wv), to_np(Wo)
    bo2 = to_np(bo).reshape(1, D)
    in_maps = [
        {
            "query": to_np(query[b]),
            "key": to_np(key[b]),
            "value": to_np(value[b]),
            "Wq": Wq, "Wk": Wk, "Wv": Wv, "Wo": Wo, "bo": bo2,
        }
        for b in range(B)
    ]
    res = run_bass_kernel_spmd(nc, in_maps, list(range(B)))
    return np.stack([res.results[b]["out"] for b in range(B)], axis=0)
